# revision 28
# baseline (speedup 1.0000x reference)
"""Trainium2 Bass kernel for nn_CollatedVanillaCNN.

The model applies a tiny CNN (log1p -> conv3x3(16->32)+bn+relu+avgpool2 ->
conv3x3(32->64)+bn+relu+avgpool2 -> fc(64->16)+bn+relu -> fc(16->8) -> expm1)
independently to the 4x4 sliding window at every pixel of x[4,16,128,128]
(zero-padded right/bottom), producing out[4,8,128,128].

Strategy: every output pixel is an independent sample => express the whole
network as dense matmul stages over pixels (features on SBUF partitions,
pixels on the free dim):

  conv1 : K=(sh,sw,c)=256 -> M=(pw,qw,o1)=512   (masked 3x3 taps, banded)
  conv2 : K=(qw,o1)=128x2 -> M=(r2,t2,o2)=256   (pool1 folded in)
  fc1   : K=(t2,o2)=128   -> M=16               (pool2 folded: r2 pre-added,
                                                 t2 in K, slot-packed PSUM)
  fc2   : K=16 -> M=8 (block-diag over 8 slots, per 8-tile group)

Input layout: instead of a full im2col (16x replication, 8MB/core), the host
uploads y2[17,128,512]: an (lo,sw)-replicated row buffer (8x, 4.4MB/core)
with partition p = 64*lo + 4*c + sw, free = 128*r' + col holding
x[c, r'+lo, col+sw].  The conv1 rhs for (tile t, chunk h) is then just the
contiguous slice y2l[:, 128*(4t+2h) : +512] - zero on-device data movement.
log1p runs once per 1024-col chunk instead of once per replicated tile.

bn scales/biases are folded into weights / activation biases.  Elementwise
work is spread across Act (ln, relu1-half0, relu2-half0, exp), DVE
(relu1-half1, relu2-half1, relu3, -1) and GpSimd (pool1 pw-folds) so the
tensor engine (12.125 matmuls / 512-px tile, ~247ns each) is the only
near-saturated engine.  A skew-3/4 software pipeline (front F(s), mid
M(s-3), back G(s-4)) gives the conv1 -> relu1 -> pwadd -> conv2 chain a
full period of slack; dummy matmuls keep the PE p-state ramped across
the fill and wind-down phases.

Sharding: pure data parallel over B x H/2: core = (b, row half), 8192 pixels
per core, 16 tiles of 512 pixels (4 image rows x 128 cols).  Host does only
data movement (pad/replicate/layout); all arithmetic runs on device.
"""

import ml_dtypes
import numpy as np

import concourse.bacc as bacc
import concourse.bass as bass
import concourse.mybir as mybir
import concourse.tile as tile
from concourse import bass_utils

AF = mybir.ActivationFunctionType
ALU = mybir.AluOpType
F32 = mybir.dt.float32
F32R = mybir.dt.float32r

EPS = 1e-5
NCORES = 8
NT = 16          # pixel tiles per core (each 4 image rows x 128 cols = 512 px)
NBLK = 17        # y2 row blocks (68 rows of 128 cols, 4 rows per block)


# ---------------------------------------------------------------- host packing

def _pack_weights(p):
    """Pack all network params into device-layout matmul weights / biases."""
    w1 = p["conv1_w"].astype(np.float64)   # [32,16,3,3]
    w2 = p["conv2_w"].astype(np.float64)   # [64,32,3,3]
    s1 = (p["bn1_g"] / np.sqrt(p["bn1_v"] + EPS)).astype(np.float64)
    s2 = (p["bn2_g"] / np.sqrt(p["bn2_v"] + EPS)).astype(np.float64)
    s3 = (p["bn3_g"] / np.sqrt(p["bn3_v"] + EPS)).astype(np.float64)

    # conv1: rows (lo,c,sw) p=64lo+4c+sw, cols f=(pw*4+qw)*32+o, chunks h
    W1 = np.zeros((2, 128, 512), np.float64)
    pp = np.arange(128)
    c_of_p = (pp % 64) // 4
    sw_of_p = pp % 4
    f = np.arange(512)
    pw_of_f = f // 128
    qw_of_f = (f % 128) // 32
    o_of_f = f % 32
    for h in range(2):
        sh = 2 * h + pp // 64                       # [128]
        du = sh[:, None] - pw_of_f[None, :] + 1      # [128,512]
        dv = sw_of_p[:, None] - qw_of_f[None, :] + 1
        valid = (du >= 0) & (du < 3) & (dv >= 0) & (dv < 3)
        duc = np.clip(du, 0, 2)
        dvc = np.clip(dv, 0, 2)
        vals = w1[o_of_f[None, :].repeat(128, 0),
                  c_of_p[:, None].repeat(512, 1),
                  duc, dvc]
        W1[h] = np.where(valid, vals, 0.0) * s1[o_of_f][None, :]
    W1 = W1.transpose(1, 0, 2).reshape(128, 1024)    # [k, h*512+f]
    bias1 = ((p["conv1_b"] - p["bn1_m"]) * s1 + p["bn1_b"])  # [32] by o
    b1 = np.tile(bias1, 4).reshape(128, 1)           # partition (qw*32+o)

    # conv2 (+pool1 qw-fold, 1/4): rows (qw*32+o1),
    # cols ff = r*256 + r2*128 + t2*64 + o2 (pw pre-folded on DVE/Pool).
    kk = np.arange(128)
    o1_k = kk % 32
    t_k = (kk // 32) // 2
    ff = np.arange(512)
    r_f = ff // 256
    r2_f = (ff % 256) // 128
    t2_f = (ff % 128) // 64
    o2_f = ff % 64
    W2 = 0.25 * w2[o2_f[None, :].repeat(128, 0),
                   o1_k[:, None].repeat(512, 1),
                   (r_f - r2_f + 1)[None, :].repeat(128, 0),
                   t_k[:, None] - t2_f[None, :] + 1] * s2[o2_f][None, :]
    bias2 = ((p["conv2_b"] - p["bn2_m"]) * s2 + p["bn2_b"])  # [64] by o2
    b2 = np.tile(bias2, 2).reshape(128, 1)           # partition (t2*64+o2)

    # fc1 (+avgpool2): rows (t2*64+o2) (r2 pre-folded on DVE), M=16.
    # Eight slot variants: variant k8 writes only output partitions
    # 16k8..16k8+16 (other cols zero); 8 accumulating matmuls pack a
    # group's 8 tiles into one PSUM bank (PE out base partition must be
    # 0/32/64, so a direct M=16 partition-offset write is not allowed).
    base3 = np.tile(0.25 * p["fc1_w"].astype(np.float64).T * s3[None, :],
                    (2, 1))                          # [128,16]
    W3 = np.zeros((128, 8 * 128), np.float64)
    for k8 in range(8):
        W3[:, 128 * k8 + 16 * k8:128 * k8 + 16 * k8 + 16] = base3
    b3v = (p["fc1_b"] - p["bn3_m"]) * s3 + p["bn3_b"]
    b3 = np.tile(b3v, 8).reshape(128, 1)             # partition (k8,m)

    # fc2 (fp32), block-diag over the 8 slots: rows (16k8+m), cols (8k8+o).
    W4 = np.zeros((128, 64), np.float64)
    for k8 in range(8):
        W4[16 * k8:16 * k8 + 16, 8 * k8:8 * k8 + 8] = \
            p["fc2_w"].astype(np.float64).T
    b4 = np.tile(p["fc2_b"], 16).reshape(128, 1)     # partition (g,k8,o)

    wtot = np.zeros((128, 1024 + 512 + 1024), np.float64)
    wtot[:, 0:1024] = W1
    wtot[:, 1024:1536] = W2
    wtot[:, 1536:2560] = W3
    btot = np.zeros((128, 4 + 64), np.float64)
    btot[:, 0:1] = b1
    btot[:, 1:2] = b2
    btot[:, 2:3] = b3
    btot[:, 3:4] = b4
    btot[:, 4:68] = W4
    return {
        "wtot": np.ascontiguousarray(wtot, np.float32),
        "btot": np.ascontiguousarray(btot, np.float32),
    }


def _y2_core(xs):
    """xs: [16, 67, 131] padded row-slab -> y2 [17 blocks, 128, 512].

    y2[b, 64*lo+4*c+sw, 128*rr+col] = xs[c, 4*b+rr+lo, col+sw]  (0 padded).
    """
    xsp = np.pad(xs, ((0, 0), (0, 2), (0, 0)))       # rows 67 -> 69
    y2 = np.zeros((68, 128, 128), np.float32)        # [r', p, col]
    for lo in range(2):
        for sw in range(4):
            parts = 64 * lo + 4 * np.arange(16) + sw
            y2[:, parts, :] = xsp[:, lo:lo + 68, sw:sw + 128].transpose(1, 0, 2)
    return np.ascontiguousarray(
        y2.reshape(17, 4, 128, 128).transpose(0, 2, 1, 3).reshape(17, 128, 512))


def _make_in_maps(inputs):
    x = np.asarray(inputs["x"], np.float32)              # [4,16,128,128]
    xp = np.pad(x, ((0, 0), (0, 0), (0, 3), (0, 3)))     # [4,16,131,131]
    packed = _pack_weights({k: np.asarray(v, np.float64) for k, v in inputs.items()
                            if k != "x"})
    in_maps = []
    for core in range(NCORES):
        b, half = divmod(core, 2)
        r0 = half * 64
        xs = xp[b, :, r0:r0 + 67, :]
        m = dict(packed)
        m["y2"] = _y2_core(xs)
        in_maps.append(m)
    return in_maps


# ---------------------------------------------------------------- device build

def _pin_act_table_set():
    """Force every activation onto natural_log_exp_and_others (has Ln, Exp,
    Relu, Copy): the default per-function greedy set choice alternates table
    sets across Ln/Relu/Exp and burns ~2.7us per ACT_TABLE_LOAD, 28x."""
    from concourse.hw_specs import get_activation_tables as orig
    keep = "natural_log_exp_and_others"

    def patched(arch):
        t = orig(arch)
        return {name: (funcs if name == keep else set())
                for name, funcs in t.items()}

    bacc.get_activation_tables = patched


def build_nc():
    _pin_act_table_set()
    nc = bacc.Bacc("TRN2", target_bir_lowering=False, debug=False,
                   num_devices=NCORES)
    y2_d = nc.dram_tensor("y2", [NBLK, 128, 512], F32, kind="ExternalInput")
    wtot_d = nc.dram_tensor("wtot", [128, 2560], F32R, kind="ExternalInput")
    btot_d = nc.dram_tensor("btot", [128, 68], F32, kind="ExternalInput")
    out_d = nc.dram_tensor("out", [8, 64, 128], F32, kind="ExternalOutput")

    with tile.TileContext(nc) as tc:
        with (
            tc.tile_pool(name="wpool", bufs=1) as wpool,
            tc.tile_pool(name="relu1", bufs=3) as relu1p,
            tc.tile_pool(name="arpool", bufs=4) as arp,
            tc.tile_pool(name="relu2", bufs=3) as relu2p,
            tc.tile_pool(name="relu3", bufs=2) as relu3p,
            tc.tile_pool(name="outsb", bufs=2) as outp,
            tc.tile_pool(name="ps1", bufs=2, space="PSUM") as ps1,
            tc.tile_pool(name="ps2", bufs=2, space="PSUM") as ps2,
            tc.tile_pool(name="psf1", bufs=1, space="PSUM") as psf1,
            tc.tile_pool(name="psf2", bufs=1, space="PSUM") as psf2,
        ):
            ws = wpool.tile([128, 2560], F32R, tag="ws")
            bs = wpool.tile([128, 68], F32, tag="bs")
            y2raw = wpool.tile([128, NBLK * 512], F32, tag="y2raw")
            y2l = wpool.tile([128, NBLK * 512], F32R, tag="y2l")
            w1s = ws[:, 0:1024]
            w2s = ws[:, 1024:1536]
            w3s = ws[:, 1536:2560]
            b1s = bs[:, 0:1]
            b2s = bs[:, 1:2]
            b3s = bs[:, 2:3]
            b4s = bs[:, 3:4]
            w4s = bs[:, 4:68]

            def dma_blk(b):
                nc.sync.dma_start(y2raw[:, 512 * b:512 * b + 512], y2_d[b])

            def ln_blk(b):
                nc.scalar.activation(y2l[:, 512 * b:512 * b + 512],
                                     y2raw[:, 512 * b:512 * b + 512],
                                     AF.Ln, bias=1.0, scale=1.0)

            dmys = wpool.tile([128, 512], F32, tag="dmy")

            def warmup():
                # Keep the PE busy during the input-DMA prologue so the HAM
                # clock gate reaches 8/8 before the first real matmul.
                # Alternate output regions so the dummies pipeline instead
                # of WAW-serializing.
                dmyp = ps1.tile([128, 1024], F32, tag="o1")
                nc.vector.memset(dmys[:], 0.0)
                for k in range(8):
                    off = 512 * (k % 2)
                    nc.tensor.matmul(dmyp[0:8, off:off + 256], dmys[:, 0:8],
                                     dmys[:, 0:256], start=True, stop=True)

            rl1s = {}
            ars = {}
            rl2s = {}

            def front_conv1(t):
                # conv1 (+bn1 scale): K=256, M=512.  Banded in (sh vs pw):
                # output chunk pw only needs window rows sh in
                # {pw-1,pw,pw+1} & [0,3], so 6 matmuls instead of 8.
                # rhs chunk h = y2l slice at row offset 4t+2h.
                r0 = y2l[:, 128 * (4 * t):128 * (4 * t) + 512]
                r1 = y2l[:, 128 * (4 * t + 2):128 * (4 * t + 2) + 512]
                rl1s[t] = relu1p.tile([128, 2048], F32R, name="rl1", tag="rl1")
                for half in range(2):           # halves: pw in {0,1} / {2,3}
                    o1 = ps1.tile([128, 1024], F32, tag="o1")
                    if half == 0:
                        # pw=0: sh{0,1} = chunk0 only
                        nc.tensor.matmul(o1[:, 0:512], w1s[:, 0:128],
                                         r0, start=True, stop=True)
                        # pw=1: sh{0,1,2}; chunk1 rows sh=3 are zero weights
                        nc.tensor.matmul(o1[:, 512:1024], w1s[:, 128:256],
                                         r0, start=True, stop=False)
                        nc.tensor.matmul(o1[:, 512:1024],
                                         w1s[:, 512 + 128:512 + 256],
                                         r1, start=False, stop=True)
                        # bn1 bias + relu, half 0 on Act
                        nc.scalar.activation(rl1s[t][:, 0:1024],
                                             o1[:], AF.Relu,
                                             bias=b1s[:], scale=1.0)
                    else:
                        # pw=2: sh{1,2,3}; chunk0 rows sh=0 are zero weights
                        nc.tensor.matmul(o1[:, 0:512],
                                         w1s[:, 256:384],
                                         r0, start=True, stop=False)
                        nc.tensor.matmul(o1[:, 0:512],
                                         w1s[:, 512 + 256:512 + 384],
                                         r1, start=False, stop=True)
                        # pw=3: sh{2,3} = chunk1 only
                        nc.tensor.matmul(o1[:, 512:1024],
                                         w1s[:, 512 + 384:512 + 512],
                                         r1, start=True, stop=True)
                        # bn1 bias + relu, half 1 on DVE
                        nc.vector.tensor_scalar(rl1s[t][:, 1024:2048],
                                                o1[:], b1s[:], 0.0,
                                                op0=ALU.add, op1=ALU.max)

            def pwadd(t, r):
                # pool1 pw-fold: A_r = rl1[:, pw=2r] + rl1[:, pw=2r+1].
                # Steady tiles: ONE GpSimd op over both r via strided views
                # (halves the op/semaphore overhead).  Latency-critical
                # first/last tiles: r=0 on GpSimd, r=1 on DVE in parallel.
                if t in (0, 1, 15):
                    rl1 = rl1s[t]
                    if r == 0:
                        ars[t] = arp.tile([128, 1024], F32R, name="ar",
                                          tag="ar")
                    eng = nc.vector if r == 1 else nc.gpsimd
                    eng.tensor_tensor(
                        ars[t][:, 512 * r:512 * r + 512],
                        rl1[:, 1024 * r:1024 * r + 512],
                        rl1[:, 1024 * r + 512:1024 * r + 1024], op=ALU.add)
                    if r == 1:
                        del rl1s[t]
                    return
                rl1 = rl1s[t]
                if r == 0:
                    ars[t] = arp.tile([128, 1024], F32R, name="ar", tag="ar")
                nc.gpsimd.tensor_tensor(
                    ars[t][:, 512 * r:512 * r + 512],
                    rl1[:, 1024 * r:1024 * r + 512],
                    rl1[:, 1024 * r + 512:1024 * r + 1024], op=ALU.add)
                if r == 1:
                    del rl1s[t]

            def mid_half(t, n):
                # conv2 (+pool1 qw-fold, +bn2 scale): M-chunk n (=r2),
                # 2 accumulating matmuls K=(qw,o1)=128 over r, then bias+relu.
                ar = ars[t]
                if n == 0:
                    rl2s[t] = relu2p.tile([128, 1024], F32R, name="rl2",
                                          tag="rl2")
                rl2 = rl2s[t]
                o2 = ps2.tile([128, 512], F32, tag="o2")
                for r in range(2):
                    nc.tensor.matmul(
                        o2[:],
                        w2s[:, 256 * r + 128 * n:256 * r + 128 * n + 128],
                        ar[:, 512 * r:512 * r + 512],
                        start=(r == 0), stop=(r == 1),
                    )
                if n == 0:
                    nc.scalar.activation(rl2[:, 0:512], o2[:], AF.Relu,
                                         bias=b2s[:], scale=1.0)
                elif t == 15:
                    # tail: Act is idle here and DVE is the relu3 engine;
                    # keep the fc1(15) chain off DVE.
                    nc.scalar.activation(rl2[:, 512:1024], o2[:], AF.Relu,
                                         bias=b2s[:], scale=1.0)
                else:
                    nc.vector.tensor_scalar(rl2[:, 512:1024],
                                            o2[:], b2s[:], 0.0,
                                            op0=ALU.add, op1=ALU.max)
                if n == 1:
                    del ars[t]

            f1banks = {}
            f2bank = psf2.tile([128, 512], F32, name="f2bank")

            def filler(n):
                # Dummy matmuls that keep the PE p-state ramped across
                # dependency stalls (fill phase / wind-down).  Alternate
                # two disjoint regions so they pipeline.
                for k in range(n):
                    off = 16 * (k % 2)
                    nc.tensor.matmul(f2bank[0:8, off:off + 16], dmys[:, 0:8],
                                     dmys[:, 0:16], start=True, stop=True)

            def back_fc1(t):
                # fc1 (+bn3 scale, +pool2): K=(t2,o2)=128, 2 accumulating
                # matmuls over the r2 free chunks (identical weights);
                # variant k8 writes only output partitions 16k8..16k8+16,
                # 8 tiles pack one PSUM bank.
                k8 = t % 8
                if k8 == 0:
                    f1banks[t // 8] = psf1.tile([128, 512], F32, name="f1",
                                                tag="f1")
                f1 = f1banks[t // 8]
                rl2 = rl2s[t]
                for h in range(2):
                    nc.tensor.matmul(f1[:], w3s[:, 128 * k8:128 * k8 + 128],
                                     rl2[:, 512 * h:512 * h + 512],
                                     start=(k8 == 0 and h == 0),
                                     stop=(k8 == 7 and h == 1))
                del rl2s[t]

            def back_fc2(g):
                # relu3 on the packed bank, then one fp32 block-diag fc2
                # matmul -> psf2 partition half 64g.  For the tail group,
                # split relu3 across DVE and Act so the halves run in
                # parallel.
                rl3 = relu3p.tile([128, 512], F32, name="rl3", tag="rl3")
                if g == 1:
                    nc.vector.tensor_scalar(rl3[0:64, :], f1banks[g][0:64, :],
                                            b3s[0:64, :], 0.0,
                                            op0=ALU.add, op1=ALU.max)
                    nc.scalar.activation(rl3[64:128, :], f1banks[g][64:128, :],
                                         AF.Relu, bias=b3s[64:128, :],
                                         scale=1.0)
                else:
                    nc.vector.tensor_scalar(rl3[:], f1banks[g][:], b3s[:], 0.0,
                                            op0=ALU.add, op1=ALU.max)
                nc.tensor.matmul(f2bank[64 * g:64 * g + 64, :], w4s,
                                 rl3[:], start=True, stop=True)
                del f1banks[g]

            def finale(g, c):
                # expm1 = exp(x + fc2_b) - 1 on this group's half, in
                # 256-px chunks so exp/sub/store pipeline.
                ob = outp.tile([64, 256], F32, name="ob", tag="ob")
                nc.scalar.activation(
                    ob[:], f2bank[64 * g:64 * g + 64, 256 * c:256 * c + 256],
                    AF.Exp, bias=b4s[64 * g:64 * g + 64, :], scale=1.0)
                ob2 = outp.tile([64, 256], F32, name="ob2", tag="ob2")
                nc.vector.tensor_scalar(ob2[:], ob[:], 1.0,
                                        None, op0=ALU.subtract)
                dst = out_d[:].copy()
                dst.ap = mybir.VecI64Pair(
                    [(512, 8), (8192, 8), (128, 2), (1, 128)])
                dst.offset = 4096 * g + 256 * c
                (nc.sync if c == 0 else nc.scalar).dma_start(dst, ob2[:])

            # Prologue: biases + W1 on Vector queue, first blocks split
            # across Sync/GpSimd queues, warmup, ln blocks 0-1.
            dma_blk(0)
            dma_blk(1)
            nc.scalar.dma_start(bs[:], btot_d[:])
            nc.scalar.dma_start(ws[:, 0:1024], wtot_d[:, 0:1024])
            warmup()
            dma_blk(2)
            dma_blk(3)
            dma_blk(4)
            dma_blk(5)
            nc.sync.dma_start(ws[:, 1024:2560], wtot_d[:, 1024:2560])
            ln_blk(0)
            ln_blk(1)
            ln_blk(2)

            # Skew-3/4 pipeline: front F(s), mid M(s-3), back G(s-4) so the
            # conv1 -> relu1 -> pwadd -> conv2 chain has a full period of
            # slack before conv2 consumes ar, and relu2 -> fc1 likewise.
            for s in range(NT):
                t0, t1, t2 = s, s - 3, s - 4
                for b in range(6 + 3 * s, min(9 + 3 * s, NBLK)):
                    dma_blk(b)
                if s in (3, 4):
                    # Fill phase: the first mids wait on the conv1->relu1->
                    # pwadd chain of tiles 0/1; run this step's front first
                    # so the PE stays busy through that latency (its ln
                    # dependency was satisfied steps ago).
                    front_conv1(t0)
                    pwadd(t0, 0)
                    pwadd(t0, 1)
                if 0 <= t1:
                    mid_half(t1, 0)
                if 0 <= t2:
                    back_fc1(t2)
                if 0 <= t1:
                    mid_half(t1, 1)
                if t2 == 7:
                    back_fc2(0)
                    finale(0, 0)
                if t2 == 8:
                    finale(0, 1)
                if s not in (3, 4):
                    front_conv1(t0)
                    if s in (0, 1, 2):
                        filler(6)
                    pwadd(t0, 0)
                    pwadd(t0, 1)
                if s + 3 < NBLK:
                    ln_blk(s + 3)

            # Compressed wind-down in tight causal order; semaphores pace
            # the chains, fillers keep the PE p-state up.
            mid_half(13, 0)
            mid_half(13, 1)
            back_fc1(12)
            mid_half(14, 0)
            mid_half(14, 1)
            back_fc1(13)
            filler(2)
            mid_half(15, 0)
            mid_half(15, 1)
            back_fc1(14)
            filler(2)
            back_fc1(15)
            filler(4)
            back_fc2(1)
            finale(1, 0)
            finale(1, 1)

    nc.compile()
    return nc


_NC = None


def _get_nc():
    global _NC
    if _NC is None:
        _NC = build_nc()
    return _NC


def _assemble(results):
    out = np.empty((4, 8, 128, 128), np.float32)
    for core in range(NCORES):
        b, half = divmod(core, 2)
        out[b, :, half * 64:half * 64 + 64, :] = results[core]["out"]
    return out


def kernel(_trace=False, **inputs):
    nc = _get_nc()
    in_maps = _make_in_maps(inputs)
    res = bass_utils.run_bass_kernel_spmd(
        nc, in_maps, core_ids=list(range(NCORES)), trace=_trace)
    out = _assemble(res.results)
    if _trace:
        return out, res
    return out


def kernel_sim(cores=None, **inputs):
    from concourse.bass_interp import CoreSim
    nc = _get_nc()
    in_maps = _make_in_maps(inputs)
    outs = []
    for core in (cores if cores is not None else range(NCORES)):
        sim = CoreSim(nc, trace=False, require_finite=False,
                      require_nnan=False)
        for k, v in in_maps[core].items():
            sim.tensor(k)[:] = v
        sim.simulate()
        outs.append({"out": sim.tensor("out").copy()})
    return outs
